# revision 18
# baseline (speedup 1.0000x reference)
"""Trainium2 Bass kernel for CantorGlobalAttention (sparse routed attention).

Strategy: the routes table is shared across batch and heads, so the sparse
gather-attention is reformulated as dense matmuls using a host-precomputed
route-multiplicity matrix m[s,j] = #{k: routes[s,k] = j}:

    out[s] = (sum_j m[s,j] exp(SC[s,j]) v[j]) / (sum_j m[s,j] exp(SC[s,j]))
    SC = q @ k^T / sqrt(HD)

Everything runs in a transposed layout (feature dim on partitions) so no
on-device transposes are needed anywhere:
    qkT[n,s]  = (W_qk^T x^T)              (W stationary)
    SCT[j,s]  = k^T(j-tile)^T q^T         (K=64 matmul)
    ET        = mT * exp(0.125 * SCT)     (ACT exp + DVE mult, bf16)
    o2T[c,s]  = [v|1]^T @ ET              (ones col -> softmax denom Z)
    outT      = o2T[0:64] * exp(-ln Z)    (recip via ACT ln/exp)
    y[s,n]    = outT^T @ W_proj(rows)     (per-core partial)

Sharding: 8 cores = 2 batches x 4 head-groups (4 heads each). Host sums the
4 per-batch partials and adds b_proj.
"""

import numpy as np
import ml_dtypes
from contextlib import ExitStack

import concourse.bacc as bacc
import concourse.mybir as mybir
import concourse.tile as tile
from concourse.bass import ts
from concourse.bass_utils import run_bass_kernel_spmd

bf16 = ml_dtypes.bfloat16
F32 = mybir.dt.float32
BF16 = mybir.dt.bfloat16
Alu = mybir.AluOpType
Act = mybir.ActivationFunctionType

B, S, D = 2, 2048, 1024
H, HD, K = 16, 64, 64
NCORES = 8
HG = 4            # head-groups (cores per batch)
NH = H // HG      # heads per core = 4
DH = NH * HD      # feature cols per core for q/k/v = 256
ST = S // 128     # 16 s-tiles
JT = S // 128     # 16 j-tiles
KT = D // 128     # 8 contraction tiles for the projections
SCK = 1024        # s-chunk for the attention inner loop
NSC = S // SCK    # 2

_CACHED_NC = None
_LAST_RESULTS = None


def _build_bass():
    nc = bacc.Bacc("TRN2", target_bir_lowering=False, debug=False)

    xT_d = nc.dram_tensor("xT", [128, KT, S], BF16, kind="ExternalInput")
    wqk_d = nc.dram_tensor("wqk", [128, KT, 2 * DH], BF16, kind="ExternalInput")
    wv_d = nc.dram_tensor("wv", [128, KT, DH], BF16, kind="ExternalInput")
    wproj_d = nc.dram_tensor("wproj", [128, 2, D], BF16, kind="ExternalInput")
    mt_d = nc.dram_tensor("mt", [128, JT, S], BF16, kind="ExternalInput")
    bqk_d = nc.dram_tensor("bqk", [128, 4], F32, kind="ExternalInput")
    bv_d = nc.dram_tensor("bv", [1, DH], BF16, kind="ExternalInput")
    y_d = nc.dram_tensor("y", [ST, 128, D], F32, kind="ExternalOutput")

    with tile.TileContext(nc) as tc, ExitStack() as ctx:
        cp = ctx.enter_context(tc.tile_pool(name="consts", bufs=1))

        wqk_sb = cp.tile([128, KT, 2 * DH], BF16)
        wv_sb = cp.tile([128, KT, DH], BF16)
        wproj_sb = cp.tile([128, 2, D], BF16)
        mt_sb = cp.tile([128, JT, S], BF16)
        bqk_sb = cp.tile([128, 4], F32)
        bv_sb = cp.tile([1, DH], BF16)
        ones_bf = cp.tile([1, 128], BF16)
        ones_f32 = cp.tile([128, 128], F32)
        qkT_sb = cp.tile([128, 4, S], BF16)      # nt 0,1 = qT ; nt 2,3 = kT
        vext_sb = cp.tile([128, ST, NH, HD + 1], BF16)
        outT_sb = cp.tile([128, 2, S], BF16)     # proj lhsT layout

        xtp = tc.alloc_tile_pool(name="xtp", bufs=1)
        xT_sb = xtp.tile([128, KT, S], BF16)

        # loads, roughly in first-use order
        nc.sync.dma_start(wqk_sb[:], wqk_d[:])
        for kt in range(KT):
            nc.sync.dma_start(xT_sb[:, kt, :], xT_d[:, kt, :])
        nc.sync.dma_start(wv_sb[:], wv_d[:])
        nc.sync.dma_start(bqk_sb[:], bqk_d[:])
        nc.sync.dma_start(bv_sb[:], bv_d[:])
        for jt in range(JT):
            nc.sync.dma_start(mt_sb[:, jt, :], mt_d[:, jt, :])
        nc.sync.dma_start(wproj_sb[:], wproj_d[:])

        nc.vector.memset(ones_bf[:], 1.0)
        nc.vector.memset(ones_f32[:], 1.0)
        nc.vector.memset(vext_sb[:, :, :, HD:HD + 1], 1.0)

        # ---- PE warmup: dummy matmuls during the initial DMA wait ----
        # (HAM clock-gate starts at 1.2 GHz; ~3.4us of sustained matmul
        # activity unthrottles to 2.4 GHz. Fill the input-DMA window.)
        pwarm = tc.alloc_tile_pool(name="pwarm", bufs=1, space="PSUM")
        warm = pwarm.tile([128, 128], F32, tag="warm", bufs=1)
        for _ in range(28):
            nc.tensor.matmul(warm[:], ones_f32[:], ones_f32[:],
                             start=True, stop=True, skip_group_check=True)
        pwarm.release()

        # ---- phase B (q,k of pair 0): kt-waves so matmuls start as soon as
        # each xT k-tile DMA lands, instead of waiting for the full tensor ----
        pbp = tc.alloc_tile_pool(name="pbp", bufs=1, space="PSUM")
        b02 = [(nt, sq) for sq in range(S // 512) for nt in (0, 2)]
        pqs = {g: pbp.tile([128, 512], F32, tag=f"pq_{g[0]}_{g[1]}",
                           name=f"pq_{g[0]}_{g[1]}") for g in b02}
        for kt in range(KT):
            for nt, sq in b02:
                nc.tensor.matmul(
                    pqs[(nt, sq)][:],
                    wqk_sb[:, kt, ts(nt, 128)],
                    xT_sb[:, kt, ts(sq, 512)],
                    start=(kt == 0), stop=(kt == KT - 1),
                )
        for nt, sq in b02:
            nc.vector.tensor_scalar(
                qkT_sb[:, nt, ts(sq, 512)], pqs[(nt, sq)][:],
                bqk_sb[:, nt:nt + 1], None, Alu.add)
        pbp.release()

        def emit_c(st, pv_pool):
            """v[st] in normal layout -> vext (copy on ACT: DVE is busier)"""
            pv = pv_pool.tile([128, DH], F32, tag="pv")
            for kt in range(KT):
                nc.tensor.matmul(
                    pv[:], xT_sb[:, kt, ts(st, 128)], wv_sb[:, kt, :],
                    start=(kt == 0), stop=False)
            nc.tensor.matmul(pv[:], ones_bf[:], bv_sb[:], start=False, stop=True)
            nc.scalar.copy(
                vext_sb[:, st, :, 0:HD],
                pv[:].rearrange("p (h d) -> p h d", d=HD))

        def emit_b_group(nt, sq, pool):
            # one sq chunk per call (4 calls cover one nt)
            if True:
                pq = pool.tile([128, 512], F32, tag="pq2")
                for kt in range(KT):
                    nc.tensor.matmul(
                        pq[:],
                        wqk_sb[:, kt, ts(nt, 128)],
                        xT_sb[:, kt, ts(sq, 512)],
                        start=(kt == 0), stop=(kt == KT - 1),
                    )
                nc.vector.tensor_scalar(
                    qkT_sb[:, nt, ts(sq, 512)], pq[:],
                    bqk_sb[:, nt:nt + 1], None, Alu.add)

        # ---- phase D: attention, head-pair row-packed ----
        # pair 0 carries phase C (v) in its first s-chunk and the rest of
        # phase B in its later chunks, so PE never idles and HAM stays warm.
        dwork = tc.alloc_tile_pool(name="dwork", bufs=3)
        o2pool = tc.alloc_tile_pool(name="o2pool", bufs=1)

        pscp = tc.alloc_tile_pool(name="pscp", bufs=2, space="PSUM")
        po2p = tc.alloc_tile_pool(name="po2p", bufs=1, space="PSUM")
        aux = tc.alloc_tile_pool(name="pvp", bufs=2, space="PSUM")
        aux_kind = "pv"

        def emit_norm(o2s_t, hl, scH):
            # Z row -> broadcast via K=1 matmul into a borrowed psc slot,
            # reciprocal on DVE (custom op), scale, DMA partition-shift into
            # the projection layout.
            rz = dwork.tile([64, SCK], F32, tag="rz", bufs=2)
            for h2 in range(2):
                pzb = aux.tile([64, 512], F32, tag="pzb")
                nc.tensor.matmul(
                    pzb[:], ones_f32[64:65, 0:64],
                    o2s_t[64:65, ts(h2, 512)], start=True, stop=True)
                nc.vector.reciprocal_approx_fast(out=rz[:, ts(h2, 512)],
                                                 in_=pzb[:])
            stage = dwork.tile([64, SCK], BF16, tag="stage", bufs=2)
            nc.vector.tensor_tensor(stage[:], o2s_t[0:64, :], rz[:], Alu.mult)
            p0 = (hl % 2) * 64
            nc.sync.dma_start(
                outT_sb[p0:p0 + 64, hl // 2, ts(scH, SCK)], stage[:])

        for pair in range(2):
            ha, hb = 2 * pair, 2 * pair + 1
            qT2 = qkT_sb[:, pair, :]
            kT2 = qkT_sb[:, 2 + pair, :]
            o2s = {}
            for hl in (ha, hb):
                for scH in range(NSC):
                    o2s[(hl, scH)] = o2pool.tile(
                        [HD + 1, SCK], F32, tag=f"o2s_{hl % 2}_{scH}",
                        name=f"o2s_{hl}_{scH}")
            for sc4 in range(4):                  # s chunks of 512
                scH, half = sc4 // 2, sc4 % 2
                s0 = sc4 * 512
                if sc4 == 2:  # scH0 halves complete -> normalize
                    for hl in (ha, hb):
                        emit_norm(o2s[(hl, 0)], hl, 0)
                po2_a = po2p.tile([HD + 1, 512], F32, tag="po2a")
                po2_b = po2p.tile([HD + 1, 512], F32, tag="po2b")
                for jt in range(JT):
                    psc = pscp.tile([128, SCK], F32, tag="psc")
                    nc.tensor.matmul(
                        psc[:, 0:512], kT2[0:64, ts(jt, 128)],
                        qT2[0:64, s0:s0 + 512], start=True, stop=True)
                    nc.tensor.matmul(
                        psc[:, 512:1024], kT2[64:128, ts(jt, 128)],
                        qT2[64:128, s0:s0 + 512], start=True, stop=True)
                    et = dwork.tile([128, SCK], BF16, tag="et")
                    nc.scalar.activation(et[:], psc[:], Act.Exp, scale=0.125)
                    mtsl = mt_sb[:, jt, s0:s0 + 512]
                    nc.vector.tensor_tensor(et[:, 0:512], et[:, 0:512],
                                            mtsl, Alu.mult)
                    nc.vector.tensor_tensor(et[:, 512:1024], et[:, 512:1024],
                                            mtsl, Alu.mult)
                    if pair == 0 and sc4 == 0:
                        emit_c(jt, aux)
                    if pair == 0 and sc4 == 1 and jt % 2 == 0:
                        g = jt // 2
                        emit_b_group(1 if g < 4 else 3, g % 4, aux)
                    nc.tensor.matmul(po2_a[:], vext_sb[:, jt, ha, :],
                                     et[:, 0:512],
                                     start=(jt == 0), stop=(jt == JT - 1))
                    nc.tensor.matmul(po2_b[:], vext_sb[:, jt, hb, :],
                                     et[:, 512:1024],
                                     start=(jt == 0), stop=(jt == JT - 1))
                nc.vector.tensor_copy(o2s[(ha, scH)][:, ts(half, 512)], po2_a[:])
                nc.scalar.copy(o2s[(hb, scH)][:, ts(half, 512)], po2_b[:])

                if pair == 0 and sc4 == 0:
                    # v done; swap the aux pool over to the phase-B remainder
                    aux.release()
                    aux = tc.alloc_tile_pool(name="pb2", bufs=2, space="PSUM")
                elif pair == 0 and sc4 == 1:
                    # B fully done; aux becomes the Z-broadcast pool
                    aux.release()
                    aux = tc.alloc_tile_pool(name="pzp", bufs=2, space="PSUM")

                if sc4 == 3 and pair == 0:
                    for hl in (ha, hb):
                        emit_norm(o2s[(hl, 1)], hl, 1)

        # final norms for pair 1 scH1, then tear down D pools
        for hl in (2, 3):
            emit_norm(o2s[(hl, 1)], hl, 1)
        aux.release()
        po2p.release()
        pscp.release()

        # ---- phase E: projection ----
        pep = tc.alloc_tile_pool(name="pe", bufs=3, space="PSUM")
        ystage = tc.alloc_tile_pool(name="ystage", bufs=3)

        def emit_proj(st):
            py = pep.tile([128, D], F32, tag="py")
            for it in range(2):
                for half in range(D // 512):
                    nc.tensor.matmul(
                        py[:, ts(half, 512)],
                        outT_sb[:, it, ts(st, 128)],
                        wproj_sb[:, it, ts(half, 512)],
                        start=(it == 0), stop=(it == 1))
            y_sb = ystage.tile([128, D], F32, tag="y_sb")
            nc.vector.tensor_copy(y_sb[:, 0:512], py[:, 0:512])
            nc.scalar.copy(y_sb[:, 512:1024], py[:, 512:1024])
            nc.sync.dma_start(y_d[st], y_sb[:])

        for st in range(ST):
            emit_proj(st)

        ystage.release()
        pep.release()
        o2pool.release()
        dwork.release()
        xtp.release()

    nc.compile()
    return nc


def _get_nc():
    global _CACHED_NC
    if _CACHED_NC is None:
        _CACHED_NC = _build_bass()
    return _CACHED_NC


def _prep_core_inputs(x, W_qkv, b_qkv, W_proj, routes_m_T):
    """Host-side shard prep for one (batch b, head-group hg) core."""
    maps = []
    for core in range(NCORES):
        b, hg = core // HG, core % HG
        c0 = hg * DH
        xT = np.ascontiguousarray(x[b].T).astype(bf16)            # (1024, 2048)
        wqk = np.concatenate(
            [W_qkv[:, c0:c0 + DH], W_qkv[:, D + c0:D + c0 + DH]], axis=1)
        wv = W_qkv[:, 2 * D + c0:2 * D + c0 + DH]
        bqk = np.concatenate([b_qkv[c0:c0 + DH], b_qkv[D + c0:D + c0 + DH]])
        bv = b_qkv[2 * D + c0:2 * D + c0 + DH]
        wproj = W_proj[c0:c0 + DH, :]                              # (256, 1024)
        maps.append({
            "xT": np.ascontiguousarray(xT.reshape(KT, 128, S).transpose(1, 0, 2)),
            "wqk": np.ascontiguousarray(
                wqk.astype(bf16).reshape(KT, 128, 2 * DH).transpose(1, 0, 2)),
            "wv": np.ascontiguousarray(
                wv.astype(bf16).reshape(KT, 128, DH).transpose(1, 0, 2)),
            "wproj": np.ascontiguousarray(
                wproj.astype(bf16).reshape(2, 128, D).transpose(1, 0, 2)),
            "mt": routes_m_T,
            "bqk": np.ascontiguousarray(
                bqk.astype(np.float32).reshape(4, 128).T),
            "bv": bv.astype(bf16).reshape(1, DH),
        })
    return maps


def kernel(x, W_qkv, b_qkv, W_proj, b_proj, routes):
    x = np.asarray(x, dtype=np.float32)
    W_qkv = np.asarray(W_qkv, dtype=np.float32)
    b_qkv = np.asarray(b_qkv, dtype=np.float32)
    W_proj = np.asarray(W_proj, dtype=np.float32)
    b_proj = np.asarray(b_proj, dtype=np.float32)
    r = np.clip(np.asarray(routes).astype(np.int64), 0, S - 1)

    # multiplicity matrix, uploaded transposed: mt[p, jt, s] = m[s, jt*128+p]
    m = np.zeros((S, S), dtype=np.float32)
    np.add.at(m, (np.arange(S)[:, None].repeat(K, 1).ravel(), r.ravel()), 1.0)
    mT = np.ascontiguousarray(
        m.T.astype(bf16).reshape(JT, 128, S).transpose(1, 0, 2))

    nc = _get_nc()
    in_maps = _prep_core_inputs(x, W_qkv, b_qkv, W_proj, mT)
    res = run_bass_kernel_spmd(nc, in_maps, core_ids=list(range(NCORES)))
    global _LAST_RESULTS
    _LAST_RESULTS = res

    y = np.zeros((B, S, D), dtype=np.float32)
    for core in range(NCORES):
        b = core // HG
        y[b] += res.results[core]["y"].reshape(S, D)
    y += b_proj[None, None, :]
    return y


# revision 19
# speedup vs baseline: 1.0188x; 1.0188x over previous
"""Trainium2 Bass kernel for CantorGlobalAttention (sparse routed attention).

Strategy: the routes table is shared across batch and heads, so the sparse
gather-attention is reformulated as dense matmuls using a host-precomputed
route-multiplicity matrix m[s,j] = #{k: routes[s,k] = j}:

    out[s] = (sum_j m[s,j] exp(SC[s,j]) v[j]) / (sum_j m[s,j] exp(SC[s,j]))
    SC = q @ k^T / sqrt(HD)

Everything runs in a transposed layout (feature dim on partitions) so no
on-device transposes are needed anywhere:
    qkT[n,s]  = (W_qk^T x^T)              (W stationary)
    SCT[j,s]  = k^T(j-tile)^T q^T         (K=64 matmul)
    ET        = mT * exp(0.125 * SCT)     (ACT exp + DVE mult, bf16)
    o2T[c,s]  = [v|1]^T @ ET              (ones col -> softmax denom Z)
    outT      = o2T[0:64] * exp(-ln Z)    (recip via ACT ln/exp)
    y[s,n]    = outT^T @ W_proj(rows)     (per-core partial)

Sharding: 8 cores = 2 batches x 4 head-groups (4 heads each). Host sums the
4 per-batch partials and adds b_proj.
"""

import numpy as np
import ml_dtypes
from contextlib import ExitStack

import concourse.bacc as bacc
import concourse.mybir as mybir
import concourse.tile as tile
from concourse.bass import ts
from concourse.bass_utils import run_bass_kernel_spmd

bf16 = ml_dtypes.bfloat16
F32 = mybir.dt.float32
BF16 = mybir.dt.bfloat16
Alu = mybir.AluOpType
Act = mybir.ActivationFunctionType

B, S, D = 2, 2048, 1024
H, HD, K = 16, 64, 64
NCORES = 8
HG = 4            # head-groups (cores per batch)
NH = H // HG      # heads per core = 4
DH = NH * HD      # feature cols per core for q/k/v = 256
ST = S // 128     # 16 s-tiles
JT = S // 128     # 16 j-tiles
KT = D // 128     # 8 contraction tiles for the projections
SCK = 1024        # s-chunk for the attention inner loop
NSC = S // SCK    # 2

_CACHED_NC = None
_LAST_RESULTS = None


def _build_bass():
    nc = bacc.Bacc("TRN2", target_bir_lowering=False, debug=False)

    xT_d = nc.dram_tensor("xT", [128, KT, S], BF16, kind="ExternalInput")
    wqk_d = nc.dram_tensor("wqk", [128, KT, 2 * DH], BF16, kind="ExternalInput")
    wv_d = nc.dram_tensor("wv", [128, KT, DH], BF16, kind="ExternalInput")
    wproj_d = nc.dram_tensor("wproj", [128, 2, D], BF16, kind="ExternalInput")
    mt_d = nc.dram_tensor("mt", [128, JT, S], BF16, kind="ExternalInput")
    bqk_d = nc.dram_tensor("bqk", [128, 4], F32, kind="ExternalInput")
    bv_d = nc.dram_tensor("bv", [1, DH], BF16, kind="ExternalInput")
    y_d = nc.dram_tensor("y", [ST, 128, D], F32, kind="ExternalOutput")

    with tile.TileContext(nc) as tc, ExitStack() as ctx:
        cp = ctx.enter_context(tc.tile_pool(name="consts", bufs=1))

        wqk_sb = cp.tile([128, KT, 2 * DH], BF16)
        wv_sb = cp.tile([128, KT, DH], BF16)
        wproj_sb = cp.tile([128, 2, D], BF16)
        mt_sb = cp.tile([128, JT, S], BF16)
        bqk_sb = cp.tile([128, 4], F32)
        bv_sb = cp.tile([1, DH], BF16)
        ones_bf = cp.tile([1, 128], BF16)
        ones_f32 = cp.tile([128, 128], F32)
        qkT_sb = cp.tile([128, 4, S], BF16)      # nt 0,1 = qT ; nt 2,3 = kT
        vext_sb = cp.tile([128, ST, NH, HD + 1], BF16)
        outT_sb = cp.tile([128, 2, S], BF16)     # proj lhsT layout

        xtp = tc.alloc_tile_pool(name="xtp", bufs=1)
        xT_sb = xtp.tile([128, KT, S], BF16)

        # loads, roughly in first-use order
        nc.sync.dma_start(wqk_sb[:], wqk_d[:])
        for kt in range(KT):
            nc.sync.dma_start(xT_sb[:, kt, :], xT_d[:, kt, :])
        nc.sync.dma_start(wv_sb[:], wv_d[:])
        nc.sync.dma_start(bqk_sb[:], bqk_d[:])
        nc.sync.dma_start(bv_sb[:], bv_d[:])
        for jt in range(JT):
            nc.sync.dma_start(mt_sb[:, jt, :], mt_d[:, jt, :])
        nc.sync.dma_start(wproj_sb[:], wproj_d[:])

        nc.vector.memset(ones_bf[:], 1.0)
        nc.vector.memset(ones_f32[:], 1.0)
        nc.vector.memset(vext_sb[:, :, :, HD:HD + 1], 1.0)

        # ---- PE warmup: dummy matmuls during the initial DMA wait ----
        # (HAM clock-gate starts at 1.2 GHz; ~3.4us of sustained matmul
        # activity unthrottles to 2.4 GHz. Fill the input-DMA window.)
        pwarm = tc.alloc_tile_pool(name="pwarm", bufs=1, space="PSUM")
        warm = pwarm.tile([128, 128], F32, tag="warm", bufs=1)
        for _ in range(28):
            nc.tensor.matmul(warm[:], ones_f32[:], ones_f32[:],
                             start=True, stop=True, skip_group_check=True)
        pwarm.release()

        # ---- phase B (q,k of pair 0): kt-waves so matmuls start as soon as
        # each xT k-tile DMA lands, instead of waiting for the full tensor ----
        pbp = tc.alloc_tile_pool(name="pbp", bufs=1, space="PSUM")
        b02 = [(nt, sq) for nt in (0, 2) for sq in range(S // 512)]
        pqs = {g: pbp.tile([128, 512], F32, tag=f"pq_{g[0]}_{g[1]}",
                           name=f"pq_{g[0]}_{g[1]}") for g in b02}
        copy_eng = 0
        for kt in range(KT):
            for nt in (0, 2):           # same stationary tile -> 4 matmuls
                for sq in range(S // 512):
                    nc.tensor.matmul(
                        pqs[(nt, sq)][:],
                        wqk_sb[:, kt, ts(nt, 128)],
                        xT_sb[:, kt, ts(sq, 512)],
                        start=(kt == 0), stop=(kt == KT - 1),
                    )
                    if kt == KT - 1:
                        # group complete: copy out immediately, alternating
                        # engines so the copies pipeline
                        if copy_eng % 2 == 0:
                            nc.vector.tensor_scalar(
                                qkT_sb[:, nt, ts(sq, 512)], pqs[(nt, sq)][:],
                                bqk_sb[:, nt:nt + 1], None, Alu.add)
                        else:
                            nc.scalar.activation(
                                qkT_sb[:, nt, ts(sq, 512)], pqs[(nt, sq)][:],
                                Act.Identity, bias=bqk_sb[:, nt:nt + 1])
                        copy_eng += 1
        pbp.release()

        def emit_c(st, pv_pool):
            """v[st] in normal layout -> vext (copy on ACT: DVE is busier)"""
            pv = pv_pool.tile([128, DH], F32, tag="pv")
            for kt in range(KT):
                nc.tensor.matmul(
                    pv[:], xT_sb[:, kt, ts(st, 128)], wv_sb[:, kt, :],
                    start=(kt == 0), stop=False)
            nc.tensor.matmul(pv[:], ones_bf[:], bv_sb[:], start=False, stop=True)
            nc.scalar.copy(
                vext_sb[:, st, :, 0:HD],
                pv[:].rearrange("p (h d) -> p h d", d=HD))

        def emit_b_group(nt, sq, pool):
            # one sq chunk per call (4 calls cover one nt)
            if True:
                pq = pool.tile([128, 512], F32, tag="pq2")
                for kt in range(KT):
                    nc.tensor.matmul(
                        pq[:],
                        wqk_sb[:, kt, ts(nt, 128)],
                        xT_sb[:, kt, ts(sq, 512)],
                        start=(kt == 0), stop=(kt == KT - 1),
                    )
                nc.vector.tensor_scalar(
                    qkT_sb[:, nt, ts(sq, 512)], pq[:],
                    bqk_sb[:, nt:nt + 1], None, Alu.add)

        # ---- phase D: attention, head-pair row-packed ----
        # pair 0 carries phase C (v) in its first s-chunk and the rest of
        # phase B in its later chunks, so PE never idles and HAM stays warm.
        dwork = tc.alloc_tile_pool(name="dwork", bufs=3)
        o2pool = tc.alloc_tile_pool(name="o2pool", bufs=1)

        pscp = tc.alloc_tile_pool(name="pscp", bufs=2, space="PSUM")
        po2p = tc.alloc_tile_pool(name="po2p", bufs=1, space="PSUM")
        aux = tc.alloc_tile_pool(name="pvp", bufs=2, space="PSUM")
        aux_kind = "pv"

        def emit_norm(o2s_t, hl, scH):
            # Z row -> broadcast via K=1 matmul into a borrowed psc slot,
            # reciprocal on DVE (custom op), scale, DMA partition-shift into
            # the projection layout.
            rz = dwork.tile([64, SCK], F32, tag="rz", bufs=2)
            for h2 in range(2):
                pzb = aux.tile([64, 512], F32, tag="pzb")
                nc.tensor.matmul(
                    pzb[:], ones_f32[64:65, 0:64],
                    o2s_t[64:65, ts(h2, 512)], start=True, stop=True)
                nc.vector.reciprocal_approx_fast(out=rz[:, ts(h2, 512)],
                                                 in_=pzb[:])
            stage = dwork.tile([64, SCK], BF16, tag="stage", bufs=2)
            nc.vector.tensor_tensor(stage[:], o2s_t[0:64, :], rz[:], Alu.mult)
            p0 = (hl % 2) * 64
            nc.sync.dma_start(
                outT_sb[p0:p0 + 64, hl // 2, ts(scH, SCK)], stage[:])

        for pair in range(2):
            ha, hb = 2 * pair, 2 * pair + 1
            qT2 = qkT_sb[:, pair, :]
            kT2 = qkT_sb[:, 2 + pair, :]
            o2s = {}
            for hl in (ha, hb):
                for scH in range(NSC):
                    o2s[(hl, scH)] = o2pool.tile(
                        [HD + 1, SCK], F32, tag=f"o2s_{hl % 2}_{scH}",
                        name=f"o2s_{hl}_{scH}")
            for sc4 in range(4):                  # s chunks of 512
                scH, half = sc4 // 2, sc4 % 2
                s0 = sc4 * 512
                if sc4 == 2 and pair == 1:  # scH0 complete -> normalize
                    for hl in (ha, hb):
                        emit_norm(o2s[(hl, 0)], hl, 0)
                po2_a = po2p.tile([HD + 1, 512], F32, tag="po2a")
                po2_b = po2p.tile([HD + 1, 512], F32, tag="po2b")
                for jt in range(JT):
                    psc = pscp.tile([128, SCK], F32, tag="psc")
                    nc.tensor.matmul(
                        psc[:, 0:512], kT2[0:64, ts(jt, 128)],
                        qT2[0:64, s0:s0 + 512], start=True, stop=True)
                    nc.tensor.matmul(
                        psc[:, 512:1024], kT2[64:128, ts(jt, 128)],
                        qT2[64:128, s0:s0 + 512], start=True, stop=True)
                    et = dwork.tile([128, SCK], BF16, tag="et")
                    nc.scalar.activation(et[:], psc[:], Act.Exp, scale=0.125)
                    mtsl = mt_sb[:, jt, s0:s0 + 512]
                    nc.vector.tensor_tensor(et[:, 0:512], et[:, 0:512],
                                            mtsl, Alu.mult)
                    nc.vector.tensor_tensor(et[:, 512:1024], et[:, 512:1024],
                                            mtsl, Alu.mult)
                    if pair == 0 and sc4 == 0:
                        emit_c(jt, aux)
                    if pair == 0 and sc4 in (1, 2) and jt % 4 == 0:
                        g = jt // 4
                        emit_b_group(1 if sc4 == 1 else 3, g, aux)
                    nc.tensor.matmul(po2_a[:], vext_sb[:, jt, ha, :],
                                     et[:, 0:512],
                                     start=(jt == 0), stop=(jt == JT - 1))
                    nc.tensor.matmul(po2_b[:], vext_sb[:, jt, hb, :],
                                     et[:, 512:1024],
                                     start=(jt == 0), stop=(jt == JT - 1))
                nc.vector.tensor_copy(o2s[(ha, scH)][:, ts(half, 512)], po2_a[:])
                nc.scalar.copy(o2s[(hb, scH)][:, ts(half, 512)], po2_b[:])

                if pair == 0 and sc4 == 0:
                    # v done; swap the aux pool over to the phase-B remainder
                    aux.release()
                    aux = tc.alloc_tile_pool(name="pb2", bufs=2, space="PSUM")
                elif pair == 0 and sc4 == 2:
                    # B fully done; aux becomes the Z-broadcast pool
                    aux.release()
                    aux = tc.alloc_tile_pool(name="pzp", bufs=2, space="PSUM")
                    for hl in (ha, hb):      # pair0 scH0 norms
                        emit_norm(o2s[(hl, 0)], hl, 0)

                if sc4 == 3 and pair == 0:
                    for hl in (ha, hb):
                        emit_norm(o2s[(hl, 1)], hl, 1)

        # final norms for pair 1 scH1, then tear down D pools
        for hl in (2, 3):
            emit_norm(o2s[(hl, 1)], hl, 1)
        aux.release()
        po2p.release()
        pscp.release()

        # ---- phase E: projection ----
        pep = tc.alloc_tile_pool(name="pe", bufs=3, space="PSUM")
        ystage = tc.alloc_tile_pool(name="ystage", bufs=3)

        def emit_proj(st):
            py = pep.tile([128, D], F32, tag="py")
            for it in range(2):
                for half in range(D // 512):
                    nc.tensor.matmul(
                        py[:, ts(half, 512)],
                        outT_sb[:, it, ts(st, 128)],
                        wproj_sb[:, it, ts(half, 512)],
                        start=(it == 0), stop=(it == 1))
            y_sb = ystage.tile([128, D], F32, tag="y_sb")
            nc.vector.tensor_copy(y_sb[:, 0:512], py[:, 0:512])
            nc.scalar.copy(y_sb[:, 512:1024], py[:, 512:1024])
            nc.sync.dma_start(y_d[st], y_sb[:])

        for st in range(ST):
            emit_proj(st)

        ystage.release()
        pep.release()
        o2pool.release()
        dwork.release()
        xtp.release()

    nc.compile()
    return nc


def _get_nc():
    global _CACHED_NC
    if _CACHED_NC is None:
        _CACHED_NC = _build_bass()
    return _CACHED_NC


def _prep_core_inputs(x, W_qkv, b_qkv, W_proj, routes_m_T):
    """Host-side shard prep for one (batch b, head-group hg) core."""
    maps = []
    for core in range(NCORES):
        b, hg = core // HG, core % HG
        c0 = hg * DH
        xT = np.ascontiguousarray(x[b].T).astype(bf16)            # (1024, 2048)
        wqk = np.concatenate(
            [W_qkv[:, c0:c0 + DH], W_qkv[:, D + c0:D + c0 + DH]], axis=1)
        wv = W_qkv[:, 2 * D + c0:2 * D + c0 + DH]
        bqk = np.concatenate([b_qkv[c0:c0 + DH], b_qkv[D + c0:D + c0 + DH]])
        bv = b_qkv[2 * D + c0:2 * D + c0 + DH]
        wproj = W_proj[c0:c0 + DH, :]                              # (256, 1024)
        maps.append({
            "xT": np.ascontiguousarray(xT.reshape(KT, 128, S).transpose(1, 0, 2)),
            "wqk": np.ascontiguousarray(
                wqk.astype(bf16).reshape(KT, 128, 2 * DH).transpose(1, 0, 2)),
            "wv": np.ascontiguousarray(
                wv.astype(bf16).reshape(KT, 128, DH).transpose(1, 0, 2)),
            "wproj": np.ascontiguousarray(
                wproj.astype(bf16).reshape(2, 128, D).transpose(1, 0, 2)),
            "mt": routes_m_T,
            "bqk": np.ascontiguousarray(
                bqk.astype(np.float32).reshape(4, 128).T),
            "bv": bv.astype(bf16).reshape(1, DH),
        })
    return maps


def kernel(x, W_qkv, b_qkv, W_proj, b_proj, routes):
    x = np.asarray(x, dtype=np.float32)
    W_qkv = np.asarray(W_qkv, dtype=np.float32)
    b_qkv = np.asarray(b_qkv, dtype=np.float32)
    W_proj = np.asarray(W_proj, dtype=np.float32)
    b_proj = np.asarray(b_proj, dtype=np.float32)
    r = np.clip(np.asarray(routes).astype(np.int64), 0, S - 1)

    # multiplicity matrix, uploaded transposed: mt[p, jt, s] = m[s, jt*128+p]
    m = np.zeros((S, S), dtype=np.float32)
    np.add.at(m, (np.arange(S)[:, None].repeat(K, 1).ravel(), r.ravel()), 1.0)
    mT = np.ascontiguousarray(
        m.T.astype(bf16).reshape(JT, 128, S).transpose(1, 0, 2))

    nc = _get_nc()
    in_maps = _prep_core_inputs(x, W_qkv, b_qkv, W_proj, mT)
    res = run_bass_kernel_spmd(nc, in_maps, core_ids=list(range(NCORES)))
    global _LAST_RESULTS
    _LAST_RESULTS = res

    y = np.zeros((B, S, D), dtype=np.float32)
    for core in range(NCORES):
        b = core // HG
        y[b] += res.results[core]["y"].reshape(S, D)
    y += b_proj[None, None, :]
    return y


# revision 21
# speedup vs baseline: 1.0396x; 1.0204x over previous
"""Trainium2 Bass kernel for CantorGlobalAttention (sparse routed attention).

Strategy: the routes table is shared across batch and heads, so the sparse
gather-attention is reformulated as dense matmuls using a host-precomputed
route-multiplicity matrix m[s,j] = #{k: routes[s,k] = j}:

    out[s] = (sum_j m[s,j] exp(SC[s,j]) v[j]) / (sum_j m[s,j] exp(SC[s,j]))
    SC = q @ k^T / sqrt(HD)

Everything runs in a transposed layout (feature dim on partitions) so no
on-device transposes are needed anywhere:
    qkT[n,s]  = (W_qk^T x^T)              (W stationary)
    SCT[j,s]  = k^T(j-tile)^T q^T         (K=64 matmul)
    ET        = mT * exp(0.125 * SCT)     (ACT exp + DVE mult, bf16)
    o2T[c,s]  = [v|1]^T @ ET              (ones col -> softmax denom Z)
    outT      = o2T[0:64] * exp(-ln Z)    (recip via ACT ln/exp)
    y[s,n]    = outT^T @ W_proj(rows)     (per-core partial)

Sharding: 8 cores = 2 batches x 4 head-groups (4 heads each). Host sums the
4 per-batch partials and adds b_proj.
"""

import numpy as np
import ml_dtypes
from contextlib import ExitStack

import concourse.bacc as bacc
import concourse.mybir as mybir
import concourse.tile as tile
from concourse.bass import ts
from concourse.bass_utils import run_bass_kernel_spmd

bf16 = ml_dtypes.bfloat16
F32 = mybir.dt.float32
BF16 = mybir.dt.bfloat16
Alu = mybir.AluOpType
Act = mybir.ActivationFunctionType

B, S, D = 2, 2048, 1024
H, HD, K = 16, 64, 64
NCORES = 8
HG = 4            # head-groups (cores per batch)
NH = H // HG      # heads per core = 4
DH = NH * HD      # feature cols per core for q/k/v = 256
ST = S // 128     # 16 s-tiles
JT = S // 128     # 16 j-tiles
KT = D // 128     # 8 contraction tiles for the projections
SCK = 1024        # s-chunk for the attention inner loop
NSC = S // SCK    # 2

_CACHED_NC = None
_LAST_RESULTS = None


def _build_bass():
    nc = bacc.Bacc("TRN2", target_bir_lowering=False, debug=False)

    xT_d = nc.dram_tensor("xT", [128, KT, S], BF16, kind="ExternalInput")
    wqk_d = nc.dram_tensor("wqk", [128, KT, 3 * DH], BF16, kind="ExternalInput")
    wproj_d = nc.dram_tensor("wproj", [128, 2, D], BF16, kind="ExternalInput")
    mt_d = nc.dram_tensor("mt", [128, JT, S], BF16, kind="ExternalInput")
    bqk_d = nc.dram_tensor("bqk", [128, 6], F32, kind="ExternalInput")
    y_d = nc.dram_tensor("y", [ST, 128, D], F32, kind="ExternalOutput")

    with tile.TileContext(nc) as tc, ExitStack() as ctx:
        cp = ctx.enter_context(tc.tile_pool(name="consts", bufs=1))

        wqk_sb = cp.tile([128, KT, 3 * DH], BF16)
        wproj_sb = cp.tile([128, 2, D], BF16)
        mt_sb = cp.tile([128, JT, S], BF16)
        bqk_sb = cp.tile([128, 6], F32)
        ones_f32 = cp.tile([128, 128], F32)
        qkT_sb = cp.tile([128, 6, S], BF16)  # nt 0,1 = qT; 2,3 = kT; 4,5 = vT
        vext_sb = cp.tile([128, ST, NH, HD + 1], BF16)
        outT_sb = cp.tile([128, 2, S], BF16)     # proj lhsT layout

        xtp = tc.alloc_tile_pool(name="xtp", bufs=1)
        xT_sb = xtp.tile([128, KT, S], BF16)

        # loads, roughly in first-use order
        nc.sync.dma_start(wqk_sb[:], wqk_d[:])
        for kt in range(KT):
            nc.sync.dma_start(xT_sb[:, kt, :], xT_d[:, kt, :])
        nc.sync.dma_start(bqk_sb[:], bqk_d[:])
        for jt in range(JT):
            nc.sync.dma_start(mt_sb[:, jt, :], mt_d[:, jt, :])
        nc.sync.dma_start(wproj_sb[:], wproj_d[:])

        nc.vector.memset(ones_f32[:], 1.0)
        nc.vector.memset(vext_sb[:, :, :, HD:HD + 1], 1.0)

        # ---- PE warmup: dummy matmuls during the initial DMA wait ----
        # (HAM clock-gate starts at 1.2 GHz; ~3.4us of sustained matmul
        # activity unthrottles to 2.4 GHz. Fill the input-DMA window.)
        pwarm = tc.alloc_tile_pool(name="pwarm", bufs=1, space="PSUM")
        warm = pwarm.tile([128, 128], F32, tag="warm", bufs=1)
        for _ in range(28):
            nc.tensor.matmul(warm[:], ones_f32[:], ones_f32[:],
                             start=True, stop=True, skip_group_check=True)
        pwarm.release()

        # ---- phase B (q,k of pair 0): kt-waves so matmuls start as soon as
        # each xT k-tile DMA lands, instead of waiting for the full tensor ----
        pbp = tc.alloc_tile_pool(name="pbp", bufs=1, space="PSUM")
        b02 = [(nt, sq) for nt in (0, 2) for sq in range(S // 512)]
        pqs = {g: pbp.tile([128, 512], F32, tag=f"pq_{g[0]}_{g[1]}",
                           name=f"pq_{g[0]}_{g[1]}") for g in b02}
        copy_eng = 0
        for kt in range(KT):
            for nt in (0, 2):           # same stationary tile -> 4 matmuls
                for sq in range(S // 512):
                    nc.tensor.matmul(
                        pqs[(nt, sq)][:],
                        wqk_sb[:, kt, ts(nt, 128)],
                        xT_sb[:, kt, ts(sq, 512)],
                        start=(kt == 0), stop=(kt == KT - 1),
                    )
                    if kt == KT - 1:
                        # group complete: copy out immediately, alternating
                        # engines so the copies pipeline
                        if copy_eng % 2 == 0:
                            nc.vector.tensor_scalar(
                                qkT_sb[:, nt, ts(sq, 512)], pqs[(nt, sq)][:],
                                bqk_sb[:, nt:nt + 1], None, Alu.add)
                        else:
                            nc.scalar.activation(
                                qkT_sb[:, nt, ts(sq, 512)], pqs[(nt, sq)][:],
                                Act.Identity, bias=bqk_sb[:, nt:nt + 1])
                        copy_eng += 1
        pbp.release()

        def emit_b_pair(nt, sp, pool):
            # two sq chunks in kt-major order: one LDWEIGHTS serves two MMs
            sqs = (2 * sp, 2 * sp + 1)
            pq2 = {sq: pool.tile([128, 512], F32, tag=f"pq2_{sq % 2}",
                                 name=f"pq2_{nt}_{sq}") for sq in sqs}
            for kt in range(KT):
                for sq in sqs:
                    nc.tensor.matmul(
                        pq2[sq][:],
                        wqk_sb[:, kt, ts(nt, 128)],
                        xT_sb[:, kt, ts(sq, 512)],
                        start=(kt == 0), stop=(kt == KT - 1),
                    )
            for i, sq in enumerate(sqs):
                if i == 0:
                    nc.vector.tensor_scalar(
                        qkT_sb[:, nt, ts(sq, 512)], pq2[sq][:],
                        bqk_sb[:, nt:nt + 1], None, Alu.add)
                else:
                    nc.scalar.activation(
                        qkT_sb[:, nt, ts(sq, 512)], pq2[sq][:],
                        Act.Identity, bias=bqk_sb[:, nt:nt + 1])

        def emit_vtr(pair, jt):
            # transpose vT (both heads of the pair) into the vext layout:
            # (128,128) xbar transpose to a contiguous staging tile, then a
            # cheap DVE copy into the strided [v|1] slot
            stg = dwork.tile([128, 128], BF16, tag="stg", bufs=2)
            nc.sync.dma_start(stg[:], qkT_sb[:, 4 + pair, ts(jt, 128)],
                              transpose=True)
            nc.vector.tensor_copy(
                vext_sb[:, jt, 2 * pair:2 * pair + 2, 0:HD],
                stg[:].rearrange("p (h d) -> p h d", d=HD))

        # ---- phase D: attention, head-pair row-packed ----
        # pair 0 carries phase C (v) in its first s-chunk and the rest of
        # phase B in its later chunks, so PE never idles and HAM stays warm.
        dwork = tc.alloc_tile_pool(name="dwork", bufs=3)
        o2pool = tc.alloc_tile_pool(name="o2pool", bufs=1)

        pscp = tc.alloc_tile_pool(name="pscp", bufs=2, space="PSUM")
        po2p = tc.alloc_tile_pool(name="po2p", bufs=1, space="PSUM")
        aux = tc.alloc_tile_pool(name="pvp", bufs=1, space="PSUM")

        def emit_norm(o2s_t, hl, scH):
            # Z row -> broadcast via K=1 matmul into a borrowed psc slot,
            # reciprocal on DVE (custom op), scale, DMA partition-shift into
            # the projection layout.
            rz = dwork.tile([64, SCK], F32, tag="rz", bufs=2)
            for h2 in range(2):
                pzb = aux.tile([64, 512], F32, tag="pzb")
                nc.tensor.matmul(
                    pzb[:], ones_f32[64:65, 0:64],
                    o2s_t[64:65, ts(h2, 512)], start=True, stop=True)
                nc.vector.reciprocal_approx_fast(out=rz[:, ts(h2, 512)],
                                                 in_=pzb[:])
            stage = dwork.tile([64, SCK], BF16, tag="stage", bufs=2)
            nc.vector.tensor_tensor(stage[:], o2s_t[0:64, :], rz[:], Alu.mult)
            p0 = (hl % 2) * 64
            nc.sync.dma_start(
                outT_sb[p0:p0 + 64, hl // 2, ts(scH, SCK)], stage[:])

        for pair in range(2):
            ha, hb = 2 * pair, 2 * pair + 1
            qT2 = qkT_sb[:, pair, :]
            kT2 = qkT_sb[:, 2 + pair, :]
            o2s = {}
            for hl in (ha, hb):
                for scH in range(NSC):
                    o2s[(hl, scH)] = o2pool.tile(
                        [HD + 1, SCK], F32, tag=f"o2s_{hl % 2}_{scH}",
                        name=f"o2s_{hl}_{scH}")
            for sc4 in range(4):                  # s chunks of 512
                scH, half = sc4 // 2, sc4 % 2
                s0 = sc4 * 512
                if sc4 == 2 and pair == 1:  # scH0 complete -> normalize
                    for hl in (ha, hb):
                        emit_norm(o2s[(hl, 0)], hl, 0)
                po2_a = po2p.tile([HD + 1, 512], F32, tag="po2a")
                po2_b = po2p.tile([HD + 1, 512], F32, tag="po2b")
                for jt in range(JT):
                    if pair == 0 and jt in (0, 8):
                        # filler: v-projection chunks (sc4 0/1) and the
                        # pair-1 q/k chunks (sc4 2/3), kt-major
                        nt = (4, 5, 1, 3)[sc4]
                        emit_b_pair(nt, jt // 8, aux)
                    psc = pscp.tile([128, SCK], F32, tag="psc")
                    nc.tensor.matmul(
                        psc[:, 0:512], kT2[0:64, ts(jt, 128)],
                        qT2[0:64, s0:s0 + 512], start=True, stop=True)
                    nc.tensor.matmul(
                        psc[:, 512:1024], kT2[64:128, ts(jt, 128)],
                        qT2[64:128, s0:s0 + 512], start=True, stop=True)
                    et = dwork.tile([128, SCK], BF16, tag="et")
                    nc.scalar.activation(et[:], psc[:], Act.Exp, scale=0.125)
                    mtsl = mt_sb[:, jt, s0:s0 + 512]
                    nc.vector.tensor_tensor(et[:, 0:512], et[:, 0:512],
                                            mtsl, Alu.mult)
                    nc.vector.tensor_tensor(et[:, 512:1024], et[:, 512:1024],
                                            mtsl, Alu.mult)
                    if pair == 0 and sc4 < 2:
                        emit_vtr(sc4, jt)
                    nc.tensor.matmul(po2_a[:], vext_sb[:, jt, ha, :],
                                     et[:, 0:512],
                                     start=(jt == 0), stop=(jt == JT - 1))
                    nc.tensor.matmul(po2_b[:], vext_sb[:, jt, hb, :],
                                     et[:, 512:1024],
                                     start=(jt == 0), stop=(jt == JT - 1))
                nc.vector.tensor_copy(o2s[(ha, scH)][:, ts(half, 512)], po2_a[:])
                nc.scalar.copy(o2s[(hb, scH)][:, ts(half, 512)], po2_b[:])

                if pair == 0 and sc4 == 3:
                    # filler done; aux becomes the Z-broadcast pool
                    aux.release()
                    aux = tc.alloc_tile_pool(name="pzp", bufs=2, space="PSUM")
                    for hl in (ha, hb):      # pair0 scH0 norms
                        emit_norm(o2s[(hl, 0)], hl, 0)

                if sc4 == 3 and pair == 0:
                    for hl in (ha, hb):
                        emit_norm(o2s[(hl, 1)], hl, 1)

        # final norms for pair 1 scH1, then tear down D pools
        for hl in (2, 3):
            emit_norm(o2s[(hl, 1)], hl, 1)
        aux.release()
        po2p.release()
        pscp.release()

        # ---- phase E: projection ----
        pep = tc.alloc_tile_pool(name="pe", bufs=3, space="PSUM")
        ystage = tc.alloc_tile_pool(name="ystage", bufs=3)

        def emit_proj(st):
            py = pep.tile([128, D], F32, tag="py")
            for it in range(2):
                for half in range(D // 512):
                    nc.tensor.matmul(
                        py[:, ts(half, 512)],
                        outT_sb[:, it, ts(st, 128)],
                        wproj_sb[:, it, ts(half, 512)],
                        start=(it == 0), stop=(it == 1))
            y_sb = ystage.tile([128, D], F32, tag="y_sb")
            nc.vector.tensor_copy(y_sb[:, 0:512], py[:, 0:512])
            nc.scalar.copy(y_sb[:, 512:1024], py[:, 512:1024])
            nc.sync.dma_start(y_d[st], y_sb[:])

        for st in range(ST):
            emit_proj(st)

        ystage.release()
        pep.release()
        o2pool.release()
        dwork.release()
        xtp.release()

    nc.compile()
    return nc


def _get_nc():
    global _CACHED_NC
    if _CACHED_NC is None:
        _CACHED_NC = _build_bass()
    return _CACHED_NC


def _prep_core_inputs(x, W_qkv, b_qkv, W_proj, routes_m_T):
    """Host-side shard prep for one (batch b, head-group hg) core."""
    maps = []
    for core in range(NCORES):
        b, hg = core // HG, core % HG
        c0 = hg * DH
        xT = np.ascontiguousarray(x[b].T).astype(bf16)            # (1024, 2048)
        wqk = np.concatenate(
            [W_qkv[:, c0:c0 + DH], W_qkv[:, D + c0:D + c0 + DH],
             W_qkv[:, 2 * D + c0:2 * D + c0 + DH]], axis=1)        # (1024, 768)
        bqk = np.concatenate([b_qkv[c0:c0 + DH], b_qkv[D + c0:D + c0 + DH],
                              b_qkv[2 * D + c0:2 * D + c0 + DH]])
        wproj = W_proj[c0:c0 + DH, :]                              # (256, 1024)
        maps.append({
            "xT": np.ascontiguousarray(xT.reshape(KT, 128, S).transpose(1, 0, 2)),
            "wqk": np.ascontiguousarray(
                wqk.astype(bf16).reshape(KT, 128, 3 * DH).transpose(1, 0, 2)),
            "wproj": np.ascontiguousarray(
                wproj.astype(bf16).reshape(2, 128, D).transpose(1, 0, 2)),
            "mt": routes_m_T,
            "bqk": np.ascontiguousarray(
                bqk.astype(np.float32).reshape(6, 128).T),
        })
    return maps


def kernel(x, W_qkv, b_qkv, W_proj, b_proj, routes):
    x = np.asarray(x, dtype=np.float32)
    W_qkv = np.asarray(W_qkv, dtype=np.float32)
    b_qkv = np.asarray(b_qkv, dtype=np.float32)
    W_proj = np.asarray(W_proj, dtype=np.float32)
    b_proj = np.asarray(b_proj, dtype=np.float32)
    r = np.clip(np.asarray(routes).astype(np.int64), 0, S - 1)

    # multiplicity matrix, uploaded transposed: mt[p, jt, s] = m[s, jt*128+p]
    m = np.zeros((S, S), dtype=np.float32)
    np.add.at(m, (np.arange(S)[:, None].repeat(K, 1).ravel(), r.ravel()), 1.0)
    mT = np.ascontiguousarray(
        m.T.astype(bf16).reshape(JT, 128, S).transpose(1, 0, 2))

    nc = _get_nc()
    in_maps = _prep_core_inputs(x, W_qkv, b_qkv, W_proj, mT)
    res = run_bass_kernel_spmd(nc, in_maps, core_ids=list(range(NCORES)))
    global _LAST_RESULTS
    _LAST_RESULTS = res

    y = np.zeros((B, S, D), dtype=np.float32)
    for core in range(NCORES):
        b = core // HG
        y[b] += res.results[core]["y"].reshape(S, D)
    y += b_proj[None, None, :]
    return y
